# revision 31
# baseline (speedup 1.0000x reference)
"""BatchHardTripletLoss on 8 trn2 NeuronCores (Bass/Tile, SPMD data-parallel).

Host sorts anchors AND Gram columns by label.  Each core owns 512 sorted
anchor rows; its columns are permuted so chunk 0 = own block, chunk 1 = the
previous core's block, chunk 2 = the next core's block (wraparound for edge
cores is harmless: no shared labels).  Because a class's columns are
contiguous after sorting (max class size asserted <= 128), every positive of
a row-tile's anchors lies in a known chunk:

    tile m=0 -> chunks {own, prev};  m=1,2 -> {own};  m=3 -> {own, next}

Only those 6 (m, chunk) blocks receive a label-masking matmul.  The mask is
an exact per-core local one-hot: the core's <= 96 distinct anchor labels are
re-indexed 0..95; anchor-side carries +2 at its local label row, column-side
-2 (zero for columns whose label no core anchor shares).  Same-label pairs
land at sim - 4... x2 = sim - 4*1... concretely  +2 * -2 = -4, so the device
Gram is

    ps[i, j] = e_i . e_j - 4*[label_i == label_j]     (masked blocks only)

Shifted entries sit at <= -3.7, true negatives within |sim| <= ~0.3, the
diagonal at 1 - 4 = -3.  Therefore, per [128, 1024] PSUM chunk-pair:
  - DVE tensor_reduce(max): EXACT hardest negative (shifted positives and
    the diagonal can never win a max against any true negative).
  - DVE tensor_reduce(min) on the 6 masked 512-col sub-blocks: EXACT hardest
    positive (the diagonal at -3 only wins for anchors with no other
    positive, which the host masks invalid).
No activation/LSE machinery; the only approximation anywhere is bf16
embedding quantization (measured ~1e-4 loss impact).

Embeddings are host-L2-normalized bf16, shipped plane-packed
[128, 16 planes, 1024] (plane = quarter*4 + k_tile) so a column-quarter load
is one contiguous 8KB-per-partition DMA.  All input DMAs ride ONE HWDGE
queue in consumption order (a single queue drains in order at full
bandwidth; multi-queue variants starve the critical first quarter via SDMA
round-robin).  While the first loads land, the PE runs dummy matmuls to keep
the HAM activity window busy, so the real stream runs at 2.4 GHz throughout.

The loss is finished ON DEVICE (combine stats, relu with the SHIFT+MARGIN
bias, ones-matmul partition sum) and a single 16-byte [1, 4] partial-loss
row is DMA'd out per core: [128, few]-shaped outputs are poison (4-32 byte
descriptors trigger HBM read-modify-write, ~7us for 512 bytes).  Anchors
with no other positive self-mask (their min is the diagonal at 1-SHIFT);
the host only sums 32 floats and divides by the valid count.
"""

import os
from contextlib import ExitStack

import numpy as np
import ml_dtypes

import concourse.bass as bass
import concourse.bacc as bacc
import concourse.mybir as mybir
import concourse.tile as tile
from concourse.bass_utils import run_bass_kernel_spmd

F32 = mybir.dt.float32
BF16 = mybir.dt.bfloat16
FP8 = mybir.dt.float8e4
AF = mybir.ActivationFunctionType
ALU = mybir.AluOpType
AX = mybir.AxisListType

B, D = 4096, 512
NCORES = 8
RPC = B // NCORES            # anchor rows per core = 512
NCH = 512                    # column chunk (PSUM bank = 512 fp32)
NM = RPC // 128              # 4 row tiles per core
NN = B // NCH                # 8 column chunks
NPAIR = NN // 2              # 4 chunk pairs (one [128,1024] PSUM tile each)
KD = D // 128                # 4 bf16 k-tiles
NCODE = 96                   # local-label one-hot rows (distinct labels <= 96)
QW = 1024                    # DMA quarter width = one chunk pair
MARGIN = 0.2
SHIFT = 4.0                  # same-label shift (+2 anchor x -2 column)

# per-m column ranges in the mins tensor (see MASKED order below)
MIN_COLS = {0: (0, 2), 1: (2, 3), 2: (3, 4), 3: (4, 6)}

# (m, permuted chunk) blocks that carry the label-code matmul + min-reduce
MASKED = [(0, 0), (0, 1), (1, 0), (2, 0), (3, 0), (3, 2)]


def build_program():
    nc = bacc.Bacc("TRN2", target_bir_lowering=False, debug=False)
    ET_d = nc.declare_dram_parameter("ET", [128, 4 * NPAIR, QW], BF16, isOutput=False)
    CD_d = nc.declare_dram_parameter("CD", [NCODE, 4 * NCH], FP8, isOutput=False)
    out_d = nc.declare_dram_parameter("out", [1, NM], F32, isOutput=True)

    with tile.TileContext(nc) as tc, ExitStack() as ctx:
        big = ctx.enter_context(tc.tile_pool(name="big", bufs=1))
        codes = ctx.enter_context(tc.tile_pool(name="codes", bufs=1))
        outs = ctx.enter_context(tc.tile_pool(name="outs", bufs=1))
        const = ctx.enter_context(tc.tile_pool(name="const", bufs=1))
        psM = ctx.enter_context(tc.tile_pool(name="psM", bufs=4, space="PSUM"))

        et = big.tile([128, 4 * NPAIR, QW], BF16, tag="et")
        cd = codes.tile([NCODE, 4 * NCH], FP8, tag="cd")
        # stat layouts are m-major so the per-m combine reduces are
        # contiguous: maxs col = m*NPAIR + p (+ col 16 = last half-block),
        # mins cols per m: 0:2 / 2:3 / 3:4 / 4:6
        out_mins = outs.tile([128, len(MASKED)], F32, tag="om")
        out_maxs = outs.tile([128, NPAIR * NM + 1], F32, tag="ox")
        hn4 = outs.tile([128, NM], F32, tag="hn4")
        hp4 = outs.tile([128, NM], F32, tag="hp4")
        dif = outs.tile([128, NM], F32, tag="dif")
        pa = outs.tile([128, NM], F32, tag="pa")
        out_sb = outs.tile([1, NM], F32, tag="osb")
        ones_c = const.tile([128, 1], F32, tag="ones")

        # ALL input loads on Sync's single HWDGE ring, in consumption order:
        # one queue drains strictly in order at full aggregate bandwidth, so
        # quarter 0 lands first and each later quarter arrives well before
        # its chunk pair is reached (multi-queue variants measured SLOWER:
        # SDMA round-robins across queues and bulk traffic starves the
        # critical first quarter)
        nc.sync.dma_start(cd[:], CD_d[:, :])
        nc.sync.dma_start(et[:, 0:2, :], ET_d[:, 0:2, :])
        nc.sync.dma_start(et[:, 2:4, :], ET_d[:, 2:4, :])
        for q in range(1, NPAIR):
            nc.sync.dma_start(
                et[:, q * 4 : (q + 1) * 4, :], ET_d[:, q * 4 : (q + 1) * 4, :]
            )

        # ---- PE warmup: dummy matmuls while the first loads land — keeps
        # the PE HAM activity window busy so the real stream runs at 2.4 GHz
        # from the start; results are never read.  The warm tile is memset
        # from GpSimd, whose preamble retires earliest.  The dummy output
        # goes into a regular psM-pool tile: the pool rotation reuses it via
        # PE program order, so no bank is lost to the warmup.
        warm = const.tile([128, NCH], BF16, tag="warm")
        nc.gpsimd.memset(warm[:], 0.0)
        nc.vector.memset(ones_c[:], 1.0)
        ps_w = psM.tile([128, 2 * NCH], F32, tag="ps", name="psw")
        NWARM = 16
        for i in range(NWARM):
            nc.tensor.matmul(
                ps_w[:, 0:NCH], lhsT=warm[:, 0:128], rhs=warm[:],
                start=(i == 0), stop=(i == NWARM - 1),
            )

        # ---- main loop over chunk pairs x row tiles ------------------------
        for p in range(NPAIR):
            for m in range(NM):
                ps = psM.tile([128, 2 * NCH], F32, tag="ps", name="ps")
                for h in range(2):
                    n = 2 * p + h
                    masked = (m, n) in MASKED
                    for k in range(KD):
                        nc.tensor.matmul(
                            ps[:, h * NCH : (h + 1) * NCH],
                            lhsT=et[:, k : k + 1, bass.ts(m, 128)],
                            rhs=et[:, p * 4 + k : p * 4 + k + 1,
                                    h * NCH : (h + 1) * NCH],
                            start=(k == 0), stop=(k == KD - 1) and not masked,
                        )
                    if masked:
                        nc.tensor.matmul(
                            ps[:, h * NCH : (h + 1) * NCH],
                            lhsT=cd[:, bass.ts(m, 128)],
                            rhs=cd[:, NCH + n * NCH : NCH + (n + 1) * NCH],
                            start=False, stop=True,
                        )
                        mcol = MASKED.index((m, n))
                        nc.vector.tensor_reduce(
                            out_mins[:, mcol : mcol + 1],
                            ps[:, h * NCH : (h + 1) * NCH], AX.X, ALU.min,
                        )
                    if (p, m) == (NPAIR - 1, NM - 1):
                        fcol = m * NPAIR + p + h
                        nc.vector.tensor_reduce(
                            out_maxs[:, fcol : fcol + 1],
                            ps[:, h * NCH : (h + 1) * NCH], AX.X, ALU.max,
                        )
                col = m * NPAIR + p
                if (p, m) == (NPAIR - 1, NM - 1):
                    # final block: two half reduces, each emitted right after
                    # its half's accumulation group stops, so the first
                    # overlaps the second half's matmuls and only a 512-wide
                    # reduce remains after the last matmul
                    pass  # (emitted inside the h loop below)
                else:
                    nc.vector.tensor_reduce(
                        out_maxs[:, col : col + 1], ps[:], AX.X, ALU.max
                    )
                # as soon as the last pair of a row tile is done, fold its
                # stats and accumulate this row tile's loss contribution:
                # per_anchor = relu(hn - hp + MARGIN - SHIFT); anchors with
                # no other positive have hp = diag = 1 - SHIFT, which drives
                # relu to 0, so the invalid mask is implicit
                if p == 1 and m == NM - 1:
                    # all 6 masked mins are final after pair 1: fold the
                    # hardest-positive per row tile here, off the tail path
                    for mm in range(NM):
                        lo, hi = MIN_COLS[mm]
                        nc.vector.tensor_reduce(
                            hp4[:, mm : mm + 1], out_mins[:, lo:hi], AX.X, ALU.min
                        )
                if p == NPAIR - 1:
                    w = NPAIR + 1 if m == NM - 1 else NPAIR
                    nc.vector.tensor_reduce(
                        hn4[:, m : m + 1],
                        out_maxs[:, m * NPAIR : m * NPAIR + w], AX.X, ALU.max,
                    )
                    # per_anchor = max(hn - hp + MARGIN - SHIFT, 0), all on
                    # DVE (an ACT relu would cost two cross-engine hops)
                    nc.vector.scalar_tensor_tensor(
                        dif[:, m : m + 1], hn4[:, m : m + 1], MARGIN - SHIFT,
                        hp4[:, m : m + 1], ALU.add, ALU.subtract,
                    )
                    nc.vector.tensor_scalar_max(
                        pa[:, m : m + 1], dif[:, m : m + 1], 0.0
                    )

        # single partition-sum matmul at the very end: keeping it out of the
        # PE FIFO during the main loop is essential (a mid-loop matmul that
        # waits on the DVE/ACT chain stalls all later real matmuls)
        ps_o = psM.tile([128, 2 * NCH], F32, tag="ps", name="ps_out")
        nc.tensor.matmul(
            ps_o[0:1, 0:NM], lhsT=ones_c[:], rhs=pa[:], start=True, stop=True
        )
        nc.vector.tensor_copy(out_sb[:], ps_o[0:1, 0:NM])
        nc.sync.dma_start(out_d[:, :], out_sb[:])

    nc.compile()
    return nc


def host_prepare(embeddings, labels):
    """Sort by label, normalize->bf16, pack planes, local one-hot codes."""
    embeddings = np.asarray(embeddings, dtype=np.float32)
    labels = np.asarray(labels).astype(np.int64)

    sort_idx = np.argsort(labels, kind="stable")
    slab = labels[sort_idx]
    cnt_all = np.bincount(labels, minlength=int(labels.max()) + 1)
    assert cnt_all.max() <= 128, "label-sorted chunk-window assumption violated"

    norm = np.maximum(np.linalg.norm(embeddings, axis=1, keepdims=True), 1e-12)
    en = (embeddings / norm)[sort_idx]
    ET = np.ascontiguousarray(en.T.astype(ml_dtypes.bfloat16))   # [D, B] sorted cols

    cnt = cnt_all[slab]
    valid_sorted = ((cnt >= 2) & (cnt <= B - 1)).astype(np.float64)

    in_maps = []
    for c in range(NCORES):
        rows = slice(c * RPC, (c + 1) * RPC)
        anchor_lab = slab[rows]
        uniq, lid_anchor = np.unique(anchor_lab, return_inverse=True)
        assert len(uniq) <= NCODE, f"core {c}: {len(uniq)} distinct labels > {NCODE}"

        CD = np.zeros((NCODE, 4 * NCH), np.float32)
        CD[lid_anchor, np.arange(RPC)] = 2.0

        order = [c, (c - 1) % NCORES, (c + 1) % NCORES] + [
            j for j in range(NN) if j not in (c, (c - 1) % NCORES, (c + 1) % NCORES)
        ]
        colperm = np.concatenate(
            [np.arange(j * NCH, (j + 1) * NCH) for j in order]
        )
        mask_cols = colperm[: 3 * NCH]
        mlab = slab[mask_cols]
        pos = np.searchsorted(uniq, mlab)
        pos_c = np.minimum(pos, len(uniq) - 1)
        hit = uniq[pos_c] == mlab
        CD[pos_c[hit], NCH + np.flatnonzero(hit)] = -2.0

        ETc = ET[:, colperm]
        # [512, 4096] -> [128p, 16 planes, 1024], plane = quarter*4 + k_tile
        ETc = np.ascontiguousarray(
            ETc.reshape(KD, 128, NPAIR, QW).transpose(1, 2, 0, 3).reshape(128, 4 * NPAIR, QW)
        )
        in_maps.append(
            {
                "ET": ETc,
                "CD": np.ascontiguousarray(CD.astype(ml_dtypes.float8_e4m3)),
            }
        )
    return in_maps, valid_sorted


_prog_cache = {}


def _get_program():
    key = (B, D, RPC)
    if key not in _prog_cache:
        _prog_cache[key] = build_program()
    return _prog_cache[key]


LAST_RESULT = None


def kernel(embeddings, labels):
    global LAST_RESULT
    in_maps, valid_sorted = host_prepare(embeddings, labels)
    nc = _get_program()
    trace = bool(int(os.environ.get("TRIPLET_TRACE", "0")))
    res = run_bass_kernel_spmd(nc, in_maps, list(range(NCORES)), trace=trace)
    LAST_RESULT = res

    n_valid = max(int(valid_sorted.sum()), 1)
    loss_sum = float(sum(r["out"].astype(np.float64).sum() for r in res.results))
    return np.array(loss_sum / n_valid, dtype=np.float32)


# revision 32
# speedup vs baseline: 1.0065x; 1.0065x over previous
"""BatchHardTripletLoss on 8 trn2 NeuronCores (Bass/Tile, SPMD data-parallel).

Host sorts anchors AND Gram columns by label.  Each core owns 512 sorted
anchor rows; its columns are permuted so chunk 0 = own block, chunk 1 = the
previous core's block, chunk 2 = the next core's block (wraparound for edge
cores is harmless: no shared labels).  Because a class's columns are
contiguous after sorting (max class size asserted <= 128), every positive of
a row-tile's anchors lies in a known chunk:

    tile m=0 -> chunks {own, prev};  m=1,2 -> {own};  m=3 -> {own, next}

Only those 6 (m, chunk) blocks receive a label-masking matmul.  The mask is
an exact per-core local one-hot: the core's <= 96 distinct anchor labels are
re-indexed 0..95; anchor-side carries +2 at its local label row, column-side
-2 (zero for columns whose label no core anchor shares).  Same-label pairs
land at sim - 4... x2 = sim - 4*1... concretely  +2 * -2 = -4, so the device
Gram is

    ps[i, j] = e_i . e_j - 4*[label_i == label_j]     (masked blocks only)

Shifted entries sit at <= -3.7, true negatives within |sim| <= ~0.3, the
diagonal at 1 - 4 = -3.  Therefore, per [128, 1024] PSUM chunk-pair:
  - DVE tensor_reduce(max): EXACT hardest negative (shifted positives and
    the diagonal can never win a max against any true negative).
  - DVE tensor_reduce(min) on the 6 masked 512-col sub-blocks: EXACT hardest
    positive (the diagonal at -3 only wins for anchors with no other
    positive, which the host masks invalid).
No activation/LSE machinery; the only approximation anywhere is bf16
embedding quantization (measured ~1e-4 loss impact).

Embeddings are host-L2-normalized bf16, shipped plane-packed
[128, 16 planes, 1024] (plane = quarter*4 + k_tile) so a column-quarter load
is one contiguous 8KB-per-partition DMA.  All input DMAs ride ONE HWDGE
queue in consumption order (a single queue drains in order at full
bandwidth; multi-queue variants starve the critical first quarter via SDMA
round-robin).  While the first loads land, the PE runs dummy matmuls to keep
the HAM activity window busy, so the real stream runs at 2.4 GHz throughout.

The loss is finished ON DEVICE (combine stats, relu with the SHIFT+MARGIN
bias, ones-matmul partition sum) and a single 16-byte [1, 4] partial-loss
row is DMA'd out per core: [128, few]-shaped outputs are poison (4-32 byte
descriptors trigger HBM read-modify-write, ~7us for 512 bytes).  Anchors
with no other positive self-mask (their min is the diagonal at 1-SHIFT);
the host only sums 32 floats and divides by the valid count.
"""

import os
from contextlib import ExitStack

import numpy as np
import ml_dtypes

import concourse.bass as bass
import concourse.bacc as bacc
import concourse.mybir as mybir
import concourse.tile as tile
from concourse.bass_utils import run_bass_kernel_spmd

F32 = mybir.dt.float32
BF16 = mybir.dt.bfloat16
FP8 = mybir.dt.float8e4
AF = mybir.ActivationFunctionType
ALU = mybir.AluOpType
AX = mybir.AxisListType

B, D = 4096, 512
NCORES = 8
RPC = B // NCORES            # anchor rows per core = 512
NCH = 512                    # column chunk (PSUM bank = 512 fp32)
NM = RPC // 128              # 4 row tiles per core
NN = B // NCH                # 8 column chunks
NPAIR = NN // 2              # 4 chunk pairs (one [128,1024] PSUM tile each)
KD = D // 128                # 4 bf16 k-tiles
NCODE = 96                   # local-label one-hot rows (distinct labels <= 96)
QW = 1024                    # DMA quarter width = one chunk pair
MARGIN = 0.2
SHIFT = 4.0                  # same-label shift (+2 anchor x -2 column)

# per-m column ranges in the mins tensor (see MASKED order below)
MIN_COLS = {0: (0, 2), 1: (2, 3), 2: (3, 4), 3: (4, 6)}

# (m, permuted chunk) blocks that carry the label-code matmul + min-reduce
MASKED = [(0, 0), (0, 1), (1, 0), (2, 0), (3, 0), (3, 2)]


def build_program():
    nc = bacc.Bacc("TRN2", target_bir_lowering=False, debug=False)
    ET_d = nc.declare_dram_parameter("ET", [128, 4 * NPAIR, QW], BF16, isOutput=False)
    CD_d = nc.declare_dram_parameter("CD", [NCODE, 4 * NCH], FP8, isOutput=False)
    out_d = nc.declare_dram_parameter("out", [1, NM], F32, isOutput=True)

    with tile.TileContext(nc) as tc, ExitStack() as ctx:
        big = ctx.enter_context(tc.tile_pool(name="big", bufs=1))
        codes = ctx.enter_context(tc.tile_pool(name="codes", bufs=1))
        outs = ctx.enter_context(tc.tile_pool(name="outs", bufs=1))
        const = ctx.enter_context(tc.tile_pool(name="const", bufs=1))
        psM = ctx.enter_context(tc.tile_pool(name="psM", bufs=4, space="PSUM"))

        et = big.tile([128, 4 * NPAIR, QW], BF16, tag="et")
        cd = codes.tile([NCODE, 4 * NCH], FP8, tag="cd")
        # stat layouts are m-major so the per-m combine reduces are
        # contiguous: maxs col = m*NPAIR + p (+ col 16 = last half-block),
        # mins cols per m: 0:2 / 2:3 / 3:4 / 4:6
        out_mins = outs.tile([128, len(MASKED)], F32, tag="om")
        out_maxs = outs.tile([128, NPAIR * NM + 1], F32, tag="ox")
        hn4 = outs.tile([128, NM], F32, tag="hn4")
        hp4 = outs.tile([128, NM], F32, tag="hp4")
        dif = outs.tile([128, NM], F32, tag="dif")
        pa = outs.tile([128, NM], F32, tag="pa")
        out_sb = outs.tile([1, NM], F32, tag="osb")
        ones_c = const.tile([128, 1], F32, tag="ones")
        relu_b = const.tile([128, 1], F32, tag="relub")

        # ALL input loads on Sync's single HWDGE ring, in consumption order:
        # one queue drains strictly in order at full aggregate bandwidth, so
        # quarter 0 lands first and each later quarter arrives well before
        # its chunk pair is reached (multi-queue variants measured SLOWER:
        # SDMA round-robins across queues and bulk traffic starves the
        # critical first quarter)
        nc.sync.dma_start(cd[:], CD_d[:, :])
        nc.sync.dma_start(et[:, 0:2, :], ET_d[:, 0:2, :])
        nc.sync.dma_start(et[:, 2:4, :], ET_d[:, 2:4, :])
        for q in range(1, NPAIR):
            nc.sync.dma_start(
                et[:, q * 4 : (q + 1) * 4, :], ET_d[:, q * 4 : (q + 1) * 4, :]
            )

        # ---- PE warmup: dummy matmuls while the first loads land — keeps
        # the PE HAM activity window busy so the real stream runs at 2.4 GHz
        # from the start; results are never read.  The warm tile is memset
        # from GpSimd, whose preamble retires earliest.  The dummy output
        # goes into a regular psM-pool tile: the pool rotation reuses it via
        # PE program order, so no bank is lost to the warmup.
        warm = const.tile([128, NCH], BF16, tag="warm")
        nc.gpsimd.memset(warm[:], 0.0)
        nc.vector.memset(ones_c[:], 1.0)
        nc.vector.memset(relu_b[:], MARGIN - SHIFT)
        # preload ACT's Relu table early so the tail doesn't pay the
        # ACT_TABLE_LOAD; ACT is otherwise idle
        nc.scalar.activation(pa[0:1, 0:1], ones_c[0:1, 0:1], AF.Relu, bias=relu_b[0:1, 0:1])
        ps_w = psM.tile([128, 2 * NCH], F32, tag="ps", name="psw")
        NWARM = 16
        for i in range(NWARM):
            nc.tensor.matmul(
                ps_w[:, 0:NCH], lhsT=warm[:, 0:128], rhs=warm[:],
                start=(i == 0), stop=(i == NWARM - 1),
            )

        # ---- main loop over chunk pairs x row tiles ------------------------
        for p in range(NPAIR):
            for m in range(NM):
                ps = psM.tile([128, 2 * NCH], F32, tag="ps", name="ps")
                for h in range(2):
                    n = 2 * p + h
                    masked = (m, n) in MASKED
                    for k in range(KD):
                        nc.tensor.matmul(
                            ps[:, h * NCH : (h + 1) * NCH],
                            lhsT=et[:, k : k + 1, bass.ts(m, 128)],
                            rhs=et[:, p * 4 + k : p * 4 + k + 1,
                                    h * NCH : (h + 1) * NCH],
                            start=(k == 0), stop=(k == KD - 1) and not masked,
                        )
                    if masked:
                        nc.tensor.matmul(
                            ps[:, h * NCH : (h + 1) * NCH],
                            lhsT=cd[:, bass.ts(m, 128)],
                            rhs=cd[:, NCH + n * NCH : NCH + (n + 1) * NCH],
                            start=False, stop=True,
                        )
                        mcol = MASKED.index((m, n))
                        nc.vector.tensor_reduce(
                            out_mins[:, mcol : mcol + 1],
                            ps[:, h * NCH : (h + 1) * NCH], AX.X, ALU.min,
                        )
                    if (p, m) == (NPAIR - 1, NM - 1):
                        fcol = m * NPAIR + p + h
                        nc.vector.tensor_reduce(
                            out_maxs[:, fcol : fcol + 1],
                            ps[:, h * NCH : (h + 1) * NCH], AX.X, ALU.max,
                        )
                col = m * NPAIR + p
                if (p, m) == (NPAIR - 1, NM - 1):
                    # final block: two half reduces, each emitted right after
                    # its half's accumulation group stops, so the first
                    # overlaps the second half's matmuls and only a 512-wide
                    # reduce remains after the last matmul
                    pass  # (emitted inside the h loop below)
                else:
                    nc.vector.tensor_reduce(
                        out_maxs[:, col : col + 1], ps[:], AX.X, ALU.max
                    )
                # as soon as the last pair of a row tile is done, fold its
                # stats and accumulate this row tile's loss contribution:
                # per_anchor = relu(hn - hp + MARGIN - SHIFT); anchors with
                # no other positive have hp = diag = 1 - SHIFT, which drives
                # relu to 0, so the invalid mask is implicit
                if p == NPAIR - 1:
                    w = NPAIR + 1 if m == NM - 1 else NPAIR
                    nc.vector.tensor_reduce(
                        hn4[:, m : m + 1],
                        out_maxs[:, m * NPAIR : m * NPAIR + w], AX.X, ALU.max,
                    )
                    lo, hi = MIN_COLS[m]
                    nc.vector.tensor_reduce(
                        hp4[:, m : m + 1], out_mins[:, lo:hi], AX.X, ALU.min
                    )
                    nc.vector.tensor_tensor(
                        dif[:, m : m + 1], hn4[:, m : m + 1], hp4[:, m : m + 1],
                        ALU.subtract,
                    )
                    nc.scalar.activation(
                        pa[:, m : m + 1], dif[:, m : m + 1], AF.Relu,
                        bias=relu_b[:],
                    )

        # single partition-sum matmul at the very end: keeping it out of the
        # PE FIFO during the main loop is essential (a mid-loop matmul that
        # waits on the DVE/ACT chain stalls all later real matmuls)
        ps_o = psM.tile([128, 2 * NCH], F32, tag="ps", name="ps_out")
        nc.tensor.matmul(
            ps_o[0:1, 0:NM], lhsT=ones_c[:], rhs=pa[:], start=True, stop=True
        )
        nc.vector.tensor_copy(out_sb[:], ps_o[0:1, 0:NM])
        nc.sync.dma_start(out_d[:, :], out_sb[:])

    nc.compile()
    return nc


def host_prepare(embeddings, labels):
    """Sort by label, normalize->bf16, pack planes, local one-hot codes."""
    embeddings = np.asarray(embeddings, dtype=np.float32)
    labels = np.asarray(labels).astype(np.int64)

    sort_idx = np.argsort(labels, kind="stable")
    slab = labels[sort_idx]
    cnt_all = np.bincount(labels, minlength=int(labels.max()) + 1)
    assert cnt_all.max() <= 128, "label-sorted chunk-window assumption violated"

    norm = np.maximum(np.linalg.norm(embeddings, axis=1, keepdims=True), 1e-12)
    en = (embeddings / norm)[sort_idx]
    ET = np.ascontiguousarray(en.T.astype(ml_dtypes.bfloat16))   # [D, B] sorted cols

    cnt = cnt_all[slab]
    valid_sorted = ((cnt >= 2) & (cnt <= B - 1)).astype(np.float64)

    in_maps = []
    for c in range(NCORES):
        rows = slice(c * RPC, (c + 1) * RPC)
        anchor_lab = slab[rows]
        uniq, lid_anchor = np.unique(anchor_lab, return_inverse=True)
        assert len(uniq) <= NCODE, f"core {c}: {len(uniq)} distinct labels > {NCODE}"

        CD = np.zeros((NCODE, 4 * NCH), np.float32)
        CD[lid_anchor, np.arange(RPC)] = 2.0

        order = [c, (c - 1) % NCORES, (c + 1) % NCORES] + [
            j for j in range(NN) if j not in (c, (c - 1) % NCORES, (c + 1) % NCORES)
        ]
        colperm = np.concatenate(
            [np.arange(j * NCH, (j + 1) * NCH) for j in order]
        )
        mask_cols = colperm[: 3 * NCH]
        mlab = slab[mask_cols]
        pos = np.searchsorted(uniq, mlab)
        pos_c = np.minimum(pos, len(uniq) - 1)
        hit = uniq[pos_c] == mlab
        CD[pos_c[hit], NCH + np.flatnonzero(hit)] = -2.0

        ETc = ET[:, colperm]
        # [512, 4096] -> [128p, 16 planes, 1024], plane = quarter*4 + k_tile
        ETc = np.ascontiguousarray(
            ETc.reshape(KD, 128, NPAIR, QW).transpose(1, 2, 0, 3).reshape(128, 4 * NPAIR, QW)
        )
        in_maps.append(
            {
                "ET": ETc,
                "CD": np.ascontiguousarray(CD.astype(ml_dtypes.float8_e4m3)),
            }
        )
    return in_maps, valid_sorted


_prog_cache = {}


def _get_program():
    key = (B, D, RPC)
    if key not in _prog_cache:
        _prog_cache[key] = build_program()
    return _prog_cache[key]


LAST_RESULT = None


def kernel(embeddings, labels):
    global LAST_RESULT
    in_maps, valid_sorted = host_prepare(embeddings, labels)
    nc = _get_program()
    trace = bool(int(os.environ.get("TRIPLET_TRACE", "0")))
    res = run_bass_kernel_spmd(nc, in_maps, list(range(NCORES)), trace=trace)
    LAST_RESULT = res

    n_valid = max(int(valid_sorted.sum()), 1)
    loss_sum = float(sum(r["out"].astype(np.float64).sum() for r in res.results))
    return np.array(loss_sum / n_valid, dtype=np.float32)
